# revision 16
# baseline (speedup 1.0000x reference)
"""Trainium2 Bass kernel for nn_MessagePassingNN (GNN message passing).

Strategy (8 NeuronCores, SPMD):
  - Nodes sharded: core c owns nodes [c*12500, (c+1)*12500).
  - Edges sharded by owner of `second` (the scatter destination); within a
    core, edges are sorted by `second` and grouped into 128-node destination
    blocks, padded to a fixed number of 128-message tiles per block (TPB).
  - Each iteration: gather h[first] rows from a full DRAM replica
    (indirect DMA), compute msg = selu(main@Wm1 + h[second]@Wm2 + bm) where
    the second-endpoint term is expanded per destination block with a 0/1
    indicator matmul (h[second]@Wm2+bm is computed once per node: C = h@Wm2+bm).
    Segment-sum via indicator matmul into PSUM, GRU update on the core's node
    shard (feature-transposed layout), then AllGather of the updated shard
    into the next replica.
  - Final: per-core partial graph pooling via indicator matmul, AllReduce,
    3-layer MLP (redundantly on every core), output [1, 512] from core 0.

selu(x) = LAM*max(x,0) + min(LAM*ALPHA*exp(x) - LAM*ALPHA, 0), computed as
two halves r and b that are segment-summed separately (PSUM accumulates).
"""
import sys

sys.path.insert(0, "/opt/trn_rl_repo")

import numpy as np

import concourse.bass as bass
import concourse.bacc as bacc
import concourse.mybir as mybir
import concourse.tile as tile
import concourse.bass_utils as bass_utils
from concourse.masks import make_identity

F32 = mybir.dt.float32
I32 = mybir.dt.int32

N_NODES = 100000
N_CORES = 8
SH = N_NODES // N_CORES          # 12500 nodes per core
BLK = 128
NBLK = (SH + BLK - 1) // BLK     # 98 blocks
SHP = NBLK * BLK                 # 12544 padded shard size
H = 128
T_ITERS = 8
G = 512
RU = 256

LAM = 1.0507009873554805
ALPHA = 1.6732632423543772
LA = LAM * ALPHA
LNLA = float(np.log(LA))

AG_GROUPS = [list(range(N_CORES))]
DEBUG_HT = False


def _sm_row(node):
    """Global node index -> row in the shard-major padded replica."""
    return (node // SH) * SHP + (node % SH)


def _preprocess(features, first, second, graph_ids):
    """Build per-core index/segment arrays. Returns dict of host arrays."""
    first = np.asarray(first, dtype=np.int64)
    second = np.asarray(second, dtype=np.int64)
    graph_ids = np.asarray(graph_ids, dtype=np.int64)
    features = np.asarray(features, dtype=np.float32)

    owner = second // SH
    per_core = []
    counts_all = []
    for c in range(N_CORES):
        m = owner == c
        f_c = first[m]
        loc = (second[m] - c * SH).astype(np.int64)
        order = np.argsort(loc, kind="stable")
        f_c = f_c[order]
        loc = loc[order]
        blk = loc // BLK
        cnt = np.bincount(blk, minlength=NBLK)
        per_core.append((f_c, loc, blk, cnt))
        counts_all.append(cnt)
    counts_all = np.stack(counts_all)
    tpb = int(np.ceil(counts_all.max() / BLK))
    L = NBLK * tpb * BLK  # message slots per core

    gi_list, segc_list, segr_list, gid_list, h0T_list = [], [], [], [], []
    feat_sm = np.zeros((N_CORES * SHP, H), np.float32)
    for c in range(N_CORES):
        feat_sm[c * SHP : c * SHP + SH] = features[c * SH : (c + 1) * SH]
    for c in range(N_CORES):
        f_c, loc, blk, cnt = per_core[c]
        gi = np.zeros(L, np.int32)
        seg = -np.ones(L, np.float32)
        starts = np.concatenate([[0], np.cumsum(cnt)[:-1]])
        within = np.arange(len(loc)) - starts[blk]
        slot = blk * (tpb * BLK) + within
        gi[slot] = _sm_row(f_c).astype(np.int32)
        seg[slot] = (loc - blk * BLK).astype(np.float32)
        gi_list.append(np.ascontiguousarray(gi.reshape(-1, BLK).T))    # [128, NBLK*tpb]
        segc_list.append(np.ascontiguousarray(seg.reshape(-1, BLK).T))  # [128, NBLK*tpb]
        segr_list.append(seg.reshape(1, L).copy())                      # [1, L]
        gid = -np.ones(SHP, np.float32)
        gid[:SH] = graph_ids[c * SH : (c + 1) * SH].astype(np.float32)
        gid_list.append(np.ascontiguousarray(gid.reshape(NBLK, BLK).T))  # [128, NBLK]
        h0T_list.append(np.ascontiguousarray(feat_sm[c * SHP : (c + 1) * SHP].T))

    return dict(
        tpb=tpb,
        L=L,
        feat_sm=feat_sm,
        gi=gi_list,
        segc=segc_list,
        segr=segr_list,
        gid=gid_list,
        h0T=h0T_list,
    )


def _build_program(tpb, b3_val):
    """Build the Bass program. Returns (nc, input_names)."""
    L = NBLK * tpb * BLK
    nc = bacc.Bacc(
        "TRN2",
        target_bir_lowering=False,
        debug=False,
        enable_asserts=False,
        num_devices=N_CORES,
    )

    # --- external tensors ---
    feat_sm = nc.dram_tensor("feat_sm", [N_CORES * SHP, H], F32, kind="ExternalInput")
    h0T_in = nc.dram_tensor("h0T", [BLK, SHP], F32, kind="ExternalInput")
    gi_in = nc.dram_tensor("gi", [BLK, NBLK * tpb], I32, kind="ExternalInput")
    segc_in = nc.dram_tensor("segc", [BLK, NBLK * tpb], F32, kind="ExternalInput")
    segr_in = nc.dram_tensor("segr", [1, L], F32, kind="ExternalInput")
    gid_in = nc.dram_tensor("gid", [BLK, NBLK], F32, kind="ExternalInput")
    wm1_in = nc.dram_tensor("wm1", [H, H], F32, kind="ExternalInput")
    wm2_in = nc.dram_tensor("wm2", [H, H], F32, kind="ExternalInput")
    bmr_in = nc.dram_tensor("bmr", [1, H], F32, kind="ExternalInput")
    wk_in = nc.dram_tensor("wk", [H, 3 * H], F32, kind="ExternalInput")
    uk_in = nc.dram_tensor("uk", [H, 3 * H], F32, kind="ExternalInput")
    bkc_in = nc.dram_tensor("bkc", [BLK, 4], F32, kind="ExternalInput")
    w1_in = nc.dram_tensor("w1", [H, RU], F32, kind="ExternalInput")
    w2_in = nc.dram_tensor("w2", [RU, RU], F32, kind="ExternalInput")
    w3_in = nc.dram_tensor("w3", [RU, 1], F32, kind="ExternalInput")
    b1r_in = nc.dram_tensor("b1r", [BLK, 2], F32, kind="ExternalInput")
    b1e_in = nc.dram_tensor("b1e", [BLK, 2], F32, kind="ExternalInput")
    b2r_in = nc.dram_tensor("b2r", [BLK, 2], F32, kind="ExternalInput")
    b2e_in = nc.dram_tensor("b2e", [BLK, 2], F32, kind="ExternalInput")
    out_dram = nc.dram_tensor("out", [1, G], F32, kind="ExternalOutput")
    dbg_dram = (
        nc.dram_tensor("dbg_hT", [BLK, SHP], F32, kind="ExternalOutput")
        if DEBUG_HT
        else None
    )
    dbg_agg = (
        nc.dram_tensor("dbg_aggT", [BLK, SHP], F32, kind="ExternalOutput")
        if DEBUG_HT
        else None
    )

    with tile.TileContext(nc) as tc:
        with (
            tc.tile_pool(name="const", bufs=1) as cp,
            tc.tile_pool(name="work", bufs=3) as wp,
            tc.tile_pool(name="bigwork", bufs=2) as bwp,
            tc.tile_pool(name="final", bufs=2) as fp,
            tc.tile_pool(name="ps_small", bufs=4, space="PSUM") as psS,
            tc.tile_pool(name="ps_msg", bufs=2, space="PSUM") as psM,
            tc.tile_pool(name="ps_agg", bufs=2, space="PSUM") as psA,
            tc.tile_pool(name="dram", bufs=1, space="DRAM") as dp,
        ):
            # --- resident tiles ---
            ident = cp.tile([BLK, BLK], F32)
            make_identity(nc, ident[:])
            iota_i = cp.tile([BLK, BLK], I32)
            nc.gpsimd.iota(iota_i[:], pattern=[[1, BLK]], base=0, channel_multiplier=0)
            iota_f = cp.tile([BLK, BLK], F32)
            nc.vector.tensor_copy(iota_f[:], iota_i[:])
            iotc_i = cp.tile([BLK, 1], I32)
            nc.gpsimd.iota(iotc_i[:], pattern=[[1, 1]], base=0, channel_multiplier=1)
            iota_col = cp.tile([BLK, 1], F32)
            nc.vector.tensor_copy(iota_col[:], iotc_i[:])
            iog_i = cp.tile([BLK, G], I32)
            nc.gpsimd.iota(iog_i[:], pattern=[[1, G]], base=0, channel_multiplier=0)
            iota_g = cp.tile([BLK, G], F32)
            nc.vector.tensor_copy(iota_g[:], iog_i[:])
            ones1 = cp.tile([1, BLK], F32)
            nc.gpsimd.memset(ones1[:], 1.0)
            lnla_c = cp.tile([BLK, 1], F32)
            nc.gpsimd.memset(lnla_c[:], LNLA)

            hT = cp.tile([BLK, SHP], F32)
            nc.sync.dma_start(hT[:], h0T_in[:])
            gi_sb = cp.tile([BLK, NBLK * tpb], I32)
            nc.sync.dma_start(gi_sb[:], gi_in[:])
            segc_sb = cp.tile([BLK, NBLK * tpb], F32)
            nc.sync.dma_start(segc_sb[:], segc_in[:])
            gid_sb = cp.tile([BLK, NBLK], F32)
            nc.sync.dma_start(gid_sb[:], gid_in[:])

            wm1 = cp.tile([H, H], F32)
            nc.sync.dma_start(wm1[:], wm1_in[:])
            wm2 = cp.tile([H, H], F32)
            nc.sync.dma_start(wm2[:], wm2_in[:])
            bmr = cp.tile([1, H], F32)
            nc.sync.dma_start(bmr[:], bmr_in[:])
            wk = cp.tile([H, 3 * H], F32)
            nc.sync.dma_start(wk[:], wk_in[:])
            uk = cp.tile([H, 3 * H], F32)
            nc.sync.dma_start(uk[:], uk_in[:])
            bkc = cp.tile([BLK, 4], F32)
            nc.sync.dma_start(bkc[:], bkc_in[:])
            w1 = cp.tile([H, RU], F32)
            nc.sync.dma_start(w1[:], w1_in[:])
            w2aa = cp.tile([BLK, BLK], F32)
            nc.sync.dma_start(w2aa[:], w2_in[0:BLK, 0:BLK])
            w2ab = cp.tile([BLK, BLK], F32)
            nc.sync.dma_start(w2ab[:], w2_in[0:BLK, BLK:RU])
            w2ba = cp.tile([BLK, BLK], F32)
            nc.sync.dma_start(w2ba[:], w2_in[BLK:RU, 0:BLK])
            w2bb = cp.tile([BLK, BLK], F32)
            nc.sync.dma_start(w2bb[:], w2_in[BLK:RU, BLK:RU])
            w3a = cp.tile([BLK, 1], F32)
            nc.sync.dma_start(w3a[:], w3_in[0:BLK, :])
            w3b = cp.tile([BLK, 1], F32)
            nc.sync.dma_start(w3b[:], w3_in[BLK:RU, :])
            b1r = cp.tile([BLK, 2], F32)
            nc.sync.dma_start(b1r[:], b1r_in[:])
            b1e = cp.tile([BLK, 2], F32)
            nc.sync.dma_start(b1e[:], b1e_in[:])
            b2r = cp.tile([BLK, 2], F32)
            nc.sync.dma_start(b2r[:], b2r_in[:])
            b2e = cp.tile([BLK, 2], F32)
            nc.sync.dma_start(b2e[:], b2e_in[:])
            b3c = cp.tile([1, 1], F32)
            nc.gpsimd.memset(b3c[:], float(b3_val))

            # --- DRAM scratch ---
            repA = dp.tile([N_CORES * SHP, H], F32)
            repB = dp.tile([N_CORES * SHP, H], F32)
            shard_out = dp.tile([SHP, H], F32)
            pool_in = dp.tile([BLK, G], F32)
            pool_out = dp.tile([BLK, G], F32)

            def block_body(b, src_ap, t):
                # walrus requires a static (physical) offset AP for indirect
                # DMA: copy this block's indices to a fixed tile first.
                # HW consumes ONE offset per partition per call, so issue one
                # indirect gather per 128-message tile.
                idx_blk = wp.tile([BLK, tpb], I32, tag="idx_blk")
                nc.vector.tensor_copy(idx_blk[:], gi_sb[:, bass.ts(b, tpb)])
                gmain = bwp.tile([BLK, tpb * H], F32, tag="gmain")
                for u in range(tpb):
                    nc.gpsimd.indirect_dma_start(
                        out=gmain[:, bass.ts(u, H)],
                        out_offset=None,
                        in_=src_ap,
                        in_offset=bass.IndirectOffsetOnAxis(
                            ap=idx_blk[:, u : u + 1], axis=0
                        ),
                    )
                # indT[s, m] = (seg[m] == s) for all tpb tiles of this block
                srow = bwp.tile([1, tpb * BLK], F32, tag="srow")
                nc.sync.dma_start(srow[:], segr_in[:1, bass.ts(b, tpb * BLK)])
                segb = bwp.tile([BLK, tpb * BLK], F32, tag="segb")
                nc.gpsimd.partition_broadcast(segb[:], srow[:1, :])
                indT = bwp.tile([BLK, tpb * BLK], F32, tag="indT")
                nc.vector.tensor_scalar(
                    out=indT[:], in0=segb[:], scalar1=iota_col[:, :1], scalar2=None,
                    op0=mybir.AluOpType.is_equal,
                )
                # static-offset copy of this block of hT (walrus: no register
                # offsets on ldweights operands)
                h_in = wp.tile([BLK, BLK], F32, tag="h_in")
                nc.vector.tensor_copy(h_in[:], hT[:, bass.ts(b, BLK)])
                # C = h_blk @ Wm2 + bm  (node-term of the message MLP)
                c_ps = psS.tile([BLK, H], F32, space="PSUM", tag="ps")
                nc.tensor.matmul(c_ps[:], lhsT=h_in[:], rhs=wm2[:],
                                 start=True, stop=False)
                nc.tensor.matmul(c_ps[:], lhsT=ones1[:], rhs=bmr[:],
                                 start=False, stop=True)
                c_sb = wp.tile([BLK, H], F32, tag="c_sb")
                nc.scalar.copy(c_sb[:], c_ps[:])

                aggT_ps = psA.tile([BLK, BLK], F32, space="PSUM", tag="aggT")
                for u in range(tpb):
                    mt_ps = psS.tile([BLK, BLK], F32, space="PSUM", tag="ps")
                    nc.tensor.transpose(mt_ps[:], gmain[:, bass.ts(u, H)], ident[:])
                    mainT = wp.tile([BLK, BLK], F32, tag="mainT")
                    nc.vector.tensor_copy(mainT[:], mt_ps[:])
                    msg_ps = psM.tile([BLK, H], F32, space="PSUM", tag="msg")
                    nc.tensor.matmul(msg_ps[:], lhsT=mainT[:], rhs=wm1[:],
                                     start=True, stop=False)
                    nc.tensor.matmul(msg_ps[:], lhsT=indT[:, bass.ts(u, BLK)],
                                     rhs=c_sb[:], start=False, stop=True)
                    r_t = wp.tile([BLK, H], F32, tag="r_t")
                    nc.scalar.activation(
                        r_t[:], msg_ps[:], mybir.ActivationFunctionType.Relu,
                        scale=LAM,
                    )
                    e2_t = wp.tile([BLK, H], F32, tag="e2_t")
                    nc.scalar.activation(
                        e2_t[:], msg_ps[:], mybir.ActivationFunctionType.Exp,
                        bias=lnla_c[:, :1], scale=1.0,
                    )
                    b_t = wp.tile([BLK, H], F32, tag="b_t")
                    nc.vector.tensor_scalar(
                        out=b_t[:], in0=e2_t[:], scalar1=LA, scalar2=0.0,
                        op0=mybir.AluOpType.subtract, op1=mybir.AluOpType.min,
                    )
                    ind_ms = wp.tile([BLK, BLK], F32, tag="ind_ms")
                    nc.vector.tensor_scalar(
                        out=ind_ms[:], in0=iota_f[:],
                        scalar1=segc_sb[:, bass.ds(b * tpb + u, 1)], scalar2=None,
                        op0=mybir.AluOpType.is_equal,
                    )
                    # aggT[j, s] += r.T-free matmul: lhsT=r [m, j], rhs=ind [m, s]
                    nc.tensor.matmul(aggT_ps[:], lhsT=r_t[:], rhs=ind_ms[:],
                                     start=(u == 0), stop=False)
                    nc.tensor.matmul(aggT_ps[:], lhsT=b_t[:], rhs=ind_ms[:],
                                     start=False, stop=(u == tpb - 1))

                aggT = wp.tile([BLK, BLK], F32, tag="aggT_sb")
                nc.scalar.copy(aggT[:], aggT_ps[:])
                if dbg_agg is not None and t == 0:
                    nc.sync.dma_start(dbg_agg[:, bass.ts(b, BLK)], aggT[:])

                # --- GRU (feature-transposed layout [l, n]) ---
                h_blk = h_in[:]
                mz_ps = psS.tile([BLK, BLK], F32, space="PSUM", tag="ps")
                nc.tensor.matmul(mz_ps[:], lhsT=wk[:, 0:H], rhs=aggT[:],
                                 start=True, stop=False)
                nc.tensor.matmul(mz_ps[:], lhsT=uk[:, 0:H], rhs=h_blk,
                                 start=False, stop=True)
                zT = wp.tile([BLK, BLK], F32, tag="zT")
                nc.scalar.activation(zT[:], mz_ps[:],
                                     mybir.ActivationFunctionType.Sigmoid,
                                     bias=bkc[:, 0:1])
                mr_ps = psS.tile([BLK, BLK], F32, space="PSUM", tag="ps")
                nc.tensor.matmul(mr_ps[:], lhsT=wk[:, H : 2 * H], rhs=aggT[:],
                                 start=True, stop=False)
                nc.tensor.matmul(mr_ps[:], lhsT=uk[:, H : 2 * H], rhs=h_blk,
                                 start=False, stop=True)
                rT = wp.tile([BLK, BLK], F32, tag="rT")
                nc.scalar.activation(rT[:], mr_ps[:],
                                     mybir.ActivationFunctionType.Sigmoid,
                                     bias=bkc[:, 1:2])
                mhx_ps = psS.tile([BLK, BLK], F32, space="PSUM", tag="ps")
                nc.tensor.matmul(mhx_ps[:], lhsT=wk[:, 2 * H : 3 * H], rhs=aggT[:],
                                 start=True, stop=True)
                mhh_ps = psS.tile([BLK, BLK], F32, space="PSUM", tag="ps")
                nc.tensor.matmul(mhh_ps[:], lhsT=uk[:, 2 * H : 3 * H], rhs=h_blk,
                                 start=True, stop=True)
                t1 = wp.tile([BLK, BLK], F32, tag="t1")
                nc.vector.tensor_scalar(out=t1[:], in0=mhh_ps[:],
                                        scalar1=bkc[:, 3:4], scalar2=None,
                                        op0=mybir.AluOpType.add)
                t2 = wp.tile([BLK, BLK], F32, tag="t2")
                nc.vector.tensor_tensor(out=t2[:], in0=t1[:], in1=rT[:],
                                        op=mybir.AluOpType.mult)
                t3 = wp.tile([BLK, BLK], F32, tag="t3")
                nc.vector.tensor_tensor(out=t3[:], in0=t2[:], in1=mhx_ps[:],
                                        op=mybir.AluOpType.add)
                hhT = wp.tile([BLK, BLK], F32, tag="hhT")
                nc.scalar.activation(hhT[:], t3[:],
                                     mybir.ActivationFunctionType.Tanh,
                                     bias=bkc[:, 2:3])
                d_t = wp.tile([BLK, BLK], F32, tag="d_t")
                nc.vector.tensor_tensor(out=d_t[:], in0=h_blk, in1=hhT[:],
                                        op=mybir.AluOpType.subtract)
                e_t = wp.tile([BLK, BLK], F32, tag="e_t")
                nc.vector.tensor_tensor(out=e_t[:], in0=zT[:], in1=d_t[:],
                                        op=mybir.AluOpType.mult)
                hnT = wp.tile([BLK, BLK], F32, tag="hnT")
                nc.vector.tensor_tensor(out=hnT[:], in0=hhT[:], in1=e_t[:],
                                        op=mybir.AluOpType.add)
                nc.vector.tensor_copy(hT[:, bass.ts(b, BLK)], hnT[:])

                if t < T_ITERS - 1:
                    hn_ps = psS.tile([BLK, BLK], F32, space="PSUM", tag="ps")
                    nc.tensor.transpose(hn_ps[:], hnT[:], ident[:])
                    hn_sb = wp.tile([BLK, BLK], F32, tag="hn_sb")
                    nc.scalar.copy(hn_sb[:], hn_ps[:])
                    nc.sync.dma_start(shard_out[bass.ts(b, BLK), :], hn_sb[:])

            # --- main iterations ---
            for t in range(T_ITERS):
                if t == 0:
                    src_ap = feat_sm[:]
                elif t % 2 == 1:
                    src_ap = repA[:]
                else:
                    src_ap = repB[:]
                with tc.For_i(0, NBLK) as b:
                    block_body(b, src_ap, t)
                if t < T_ITERS - 1:
                    dst = repA if t % 2 == 0 else repB
                    nc.gpsimd.collective_compute(
                        "AllGather",
                        mybir.AluOpType.bypass,
                        replica_groups=AG_GROUPS,
                        ins=[shard_out.opt()],
                        outs=[dst.opt()],
                    )

            if dbg_dram is not None:
                nc.sync.dma_start(dbg_dram[:], hT[:])

            # --- graph pooling: pooledT[j, g] = sum_s h[s, j] * (gid[s] == g) ---
            pool_ps = psM.tile([BLK, G], F32, space="PSUM", tag="msg")
            for b in range(NBLK):
                hb_ps = psS.tile([BLK, BLK], F32, space="PSUM", tag="ps")
                nc.tensor.transpose(hb_ps[:], hT[:, bass.ts(b, BLK)], ident[:])
                hb_sb = fp.tile([BLK, BLK], F32, tag="hb_sb")
                nc.scalar.copy(hb_sb[:], hb_ps[:])
                indg = fp.tile([BLK, G], F32, tag="indg")
                nc.vector.tensor_scalar(
                    out=indg[:], in0=iota_g[:], scalar1=gid_sb[:, b : b + 1],
                    scalar2=None, op0=mybir.AluOpType.is_equal,
                )
                nc.tensor.matmul(pool_ps[:], lhsT=hb_sb[:], rhs=indg[:],
                                 start=(b == 0), stop=(b == NBLK - 1))
            pooledT = fp.tile([BLK, G], F32, tag="pooledT")
            nc.vector.tensor_copy(pooledT[:], pool_ps[:])
            nc.sync.dma_start(pool_in[:], pooledT[:])
            nc.gpsimd.collective_compute(
                "AllReduce",
                mybir.AluOpType.add,
                replica_groups=AG_GROUPS,
                ins=[pool_in.opt()],
                outs=[pool_out.opt()],
            )
            pld = fp.tile([BLK, G], F32, tag="pld")
            nc.sync.dma_start(pld[:], pool_out[:])

            # --- MLP ---
            def selu_block(x_ps, brel_col, bexp_col, tagp):
                rr = fp.tile([BLK, G], F32, tag="f_r")
                nc.scalar.activation(rr[:], x_ps[:],
                                     mybir.ActivationFunctionType.Relu,
                                     bias=brel_col, scale=LAM)
                ee = fp.tile([BLK, G], F32, tag="f_e")
                nc.scalar.activation(ee[:], x_ps[:],
                                     mybir.ActivationFunctionType.Exp,
                                     bias=bexp_col, scale=1.0)
                bb = fp.tile([BLK, G], F32, tag="f_b")
                nc.vector.tensor_scalar(
                    out=bb[:], in0=ee[:], scalar1=LA, scalar2=0.0,
                    op0=mybir.AluOpType.subtract, op1=mybir.AluOpType.min,
                )
                oo = fp.tile([BLK, G], F32, tag=f"{tagp}_o")
                nc.vector.tensor_tensor(out=oo[:], in0=rr[:], in1=bb[:],
                                        op=mybir.AluOpType.add)
                return oo

            x1 = []
            for half in range(2):
                x_ps = psM.tile([BLK, G], F32, space="PSUM", tag="msg")
                nc.tensor.matmul(x_ps[:], lhsT=w1[:, bass.ts(half, BLK)], rhs=pld[:],
                                 start=True, stop=True)
                x1.append(selu_block(x_ps, b1r[:, half : half + 1],
                                     b1e[:, half : half + 1], f"x1{half}"))
            x2 = []
            w2t = [[w2aa, w2ab], [w2ba, w2bb]]
            for half in range(2):
                x_ps = psM.tile([BLK, G], F32, space="PSUM", tag="msg")
                nc.tensor.matmul(x_ps[:], lhsT=w2t[0][half][:], rhs=x1[0][:],
                                 start=True, stop=False)
                nc.tensor.matmul(x_ps[:], lhsT=w2t[1][half][:], rhs=x1[1][:],
                                 start=False, stop=True)
                x2.append(selu_block(x_ps, b2r[:, half : half + 1],
                                     b2e[:, half : half + 1], f"x2{half}"))
            x3_ps = psS.tile([1, G], F32, space="PSUM", tag="ps")
            nc.tensor.matmul(x3_ps[:], lhsT=w3a[:], rhs=x2[0][:],
                             start=True, stop=False)
            nc.tensor.matmul(x3_ps[:], lhsT=w3b[:], rhs=x2[1][:],
                             start=False, stop=True)
            out_sb = fp.tile([1, G], F32, tag="out_sb")
            nc.scalar.activation(out_sb[:], x3_ps[:],
                                 mybir.ActivationFunctionType.Identity,
                                 bias=b3c[:1, :1])
            nc.sync.dma_start(out_dram[:], out_sb[:])

    nc.compile()
    return nc


def kernel(features, edges_topology, graph_ids, Wm, bm, Wk, Uk, bk,
           W1, b1, W2, b2, W3, b3, _trace=False):
    features = np.asarray(features, np.float32)
    Wm = np.asarray(Wm, np.float32)
    bm = np.asarray(bm, np.float32)
    Wk = np.asarray(Wk, np.float32)
    Uk = np.asarray(Uk, np.float32)
    bk = np.asarray(bk, np.float32)
    W1 = np.asarray(W1, np.float32)
    b1 = np.asarray(b1, np.float32)
    W2 = np.asarray(W2, np.float32)
    b2 = np.asarray(b2, np.float32)
    W3 = np.asarray(W3, np.float32)
    b3 = np.asarray(b3, np.float32)
    et = np.asarray(edges_topology)

    pp = _preprocess(features, et[0], et[1], graph_ids)
    tpb = pp["tpb"]

    nc = _build_program(tpb, float(b3[0]))

    bkc = np.stack(
        [
            bk[0, 0:H] + bk[1, 0:H],
            bk[0, H : 2 * H] + bk[1, H : 2 * H],
            bk[0, 2 * H : 3 * H],
            bk[1, 2 * H : 3 * H],
        ],
        axis=1,
    ).astype(np.float32)  # [128, 4]: bz, br, bhx, bhh

    b1r = np.stack([LAM * b1[0:BLK], LAM * b1[BLK:RU]], axis=1).astype(np.float32)
    b1e = np.stack([b1[0:BLK] + LNLA, b1[BLK:RU] + LNLA], axis=1).astype(np.float32)
    b2r = np.stack([LAM * b2[0:BLK], LAM * b2[BLK:RU]], axis=1).astype(np.float32)
    b2e = np.stack([b2[0:BLK] + LNLA, b2[BLK:RU] + LNLA], axis=1).astype(np.float32)

    in_maps = []
    for c in range(N_CORES):
        in_maps.append(
            {
                "feat_sm": pp["feat_sm"],
                "h0T": pp["h0T"][c],
                "gi": pp["gi"][c],
                "segc": pp["segc"][c],
                "segr": pp["segr"][c],
                "gid": pp["gid"][c],
                "wm1": np.ascontiguousarray(Wm[0:H]),
                "wm2": np.ascontiguousarray(Wm[H : 2 * H]),
                "bmr": bm.reshape(1, H),
                "wk": Wk,
                "uk": Uk,
                "bkc": bkc,
                "w1": W1,
                "w2": W2,
                "w3": W3,
                "b1r": b1r,
                "b1e": b1e,
                "b2r": b2r,
                "b2e": b2e,
            }
        )

    res = bass_utils.run_bass_kernel_spmd(
        nc, in_maps, core_ids=list(range(N_CORES)), trace=_trace
    )
    out = res.results[0]["out"].reshape(G, 1).astype(np.float32)
    kernel.last_results = res
    return out


# revision 19
# speedup vs baseline: 2.7326x; 2.7326x over previous
"""Trainium2 Bass kernel for nn_MessagePassingNN (GNN message passing).

Strategy (8 NeuronCores, SPMD):
  - Nodes sharded: core c owns nodes [c*12500, (c+1)*12500).
  - Edges sharded by owner of `second` (the scatter destination); within a
    core, edges are sorted by `second` and grouped into 128-node destination
    blocks, padded to a fixed number of 128-message tiles per block (TPB).
  - Each iteration: gather h[first] rows from a full DRAM replica
    (indirect DMA), compute msg = selu(main@Wm1 + h[second]@Wm2 + bm) where
    the second-endpoint term is expanded per destination block with a 0/1
    indicator matmul (h[second]@Wm2+bm is computed once per node: C = h@Wm2+bm).
    Segment-sum via indicator matmul into PSUM, GRU update on the core's node
    shard (feature-transposed layout), then AllGather of the updated shard
    into the next replica.
  - Final: per-core partial graph pooling via indicator matmul, AllReduce,
    3-layer MLP (redundantly on every core), output [1, 512] from core 0.

selu(x) = LAM*max(x,0) + min(LAM*ALPHA*exp(x) - LAM*ALPHA, 0), computed as
two halves r and b that are segment-summed separately (PSUM accumulates).
"""
import sys

sys.path.insert(0, "/opt/trn_rl_repo")

import numpy as np

import concourse.bass as bass
import concourse.bacc as bacc
import concourse.mybir as mybir
import concourse.tile as tile
import concourse.bass_utils as bass_utils
from concourse.masks import make_identity

F32 = mybir.dt.float32
I32 = mybir.dt.int32

N_NODES = 100000
N_CORES = 8
SH = N_NODES // N_CORES          # 12500 nodes per core
BLK = 128
NBLK = (SH + BLK - 1) // BLK     # 98 blocks
SHP = NBLK * BLK                 # 12544 padded shard size
H = 128
T_ITERS = 8
G = 512
RU = 256

LAM = 1.0507009873554805
ALPHA = 1.6732632423543772
LA = LAM * ALPHA
LNLA = float(np.log(LA))

AG_GROUPS = [list(range(N_CORES))]
DEBUG_HT = False
ABLATE = set()  # {"gather", "allgather", "msg", "gru", "stage"}
UNROLL = 2
STAGGERED = True
HINTS = ()


def _sm_row(node):
    """Global node index -> row in the shard-major padded replica."""
    return (node // SH) * SHP + (node % SH)


def _preprocess(features, first, second, graph_ids):
    """Build per-core index/segment arrays. Returns dict of host arrays."""
    first = np.asarray(first, dtype=np.int64)
    second = np.asarray(second, dtype=np.int64)
    graph_ids = np.asarray(graph_ids, dtype=np.int64)
    features = np.asarray(features, dtype=np.float32)

    owner = second // SH
    per_core = []
    counts_all = []
    for c in range(N_CORES):
        m = owner == c
        f_c = first[m]
        loc = (second[m] - c * SH).astype(np.int64)
        order = np.argsort(loc, kind="stable")
        f_c = f_c[order]
        loc = loc[order]
        blk = loc // BLK
        cnt = np.bincount(blk, minlength=NBLK)
        per_core.append((f_c, loc, blk, cnt))
        counts_all.append(cnt)
    counts_all = np.stack(counts_all)
    tpb = int(np.ceil(counts_all.max() / BLK))
    L = NBLK * tpb * BLK  # message slots per core

    gi_list, segc_list, segr_list, gid_list, h0T_list = [], [], [], [], []
    feat_sm = np.zeros((N_CORES * SHP, H), np.float32)
    for c in range(N_CORES):
        feat_sm[c * SHP : c * SHP + SH] = features[c * SH : (c + 1) * SH]
    for c in range(N_CORES):
        f_c, loc, blk, cnt = per_core[c]
        gi = np.zeros(L, np.int32)
        seg = -np.ones(L, np.float32)
        starts = np.concatenate([[0], np.cumsum(cnt)[:-1]])
        within = np.arange(len(loc)) - starts[blk]
        slot = blk * (tpb * BLK) + within
        gi[slot] = _sm_row(f_c).astype(np.int32)
        seg[slot] = (loc - blk * BLK).astype(np.float32)
        gi_list.append(np.ascontiguousarray(gi.reshape(-1, BLK).T))    # [128, NBLK*tpb]
        segc_list.append(np.ascontiguousarray(seg.reshape(-1, BLK).T))  # [128, NBLK*tpb]
        segr_list.append(seg.reshape(1, L).copy())                      # [1, L]
        gid = -np.ones(SHP, np.float32)
        gid[:SH] = graph_ids[c * SH : (c + 1) * SH].astype(np.float32)
        gid_list.append(np.ascontiguousarray(gid.reshape(NBLK, BLK).T))  # [128, NBLK]
        h0T_list.append(np.ascontiguousarray(feat_sm[c * SHP : (c + 1) * SHP].T))

    return dict(
        tpb=tpb,
        L=L,
        feat_sm=feat_sm,
        gi=gi_list,
        segc=segc_list,
        segr=segr_list,
        gid=gid_list,
        h0T=h0T_list,
    )


def _build_program(tpb, b3_val):
    """Build the Bass program. Returns (nc, input_names)."""
    L = NBLK * tpb * BLK
    nc = bacc.Bacc(
        "TRN2",
        target_bir_lowering=False,
        debug=False,
        enable_asserts=False,
        num_devices=N_CORES,
    )

    # --- external tensors ---
    feat_sm = nc.dram_tensor("feat_sm", [N_CORES * SHP, H], F32, kind="ExternalInput")
    h0T_in = nc.dram_tensor("h0T", [BLK, SHP], F32, kind="ExternalInput")
    gi_in = nc.dram_tensor("gi", [BLK, NBLK * tpb], I32, kind="ExternalInput")
    segc_in = nc.dram_tensor("segc", [BLK, NBLK * tpb], F32, kind="ExternalInput")
    segr_in = nc.dram_tensor("segr", [1, L], F32, kind="ExternalInput")
    gid_in = nc.dram_tensor("gid", [BLK, NBLK], F32, kind="ExternalInput")
    wm1_in = nc.dram_tensor("wm1", [H, H], F32, kind="ExternalInput")
    wm2_in = nc.dram_tensor("wm2", [H, H], F32, kind="ExternalInput")
    bmr_in = nc.dram_tensor("bmr", [1, H], F32, kind="ExternalInput")
    wk_in = nc.dram_tensor("wk", [H, 3 * H], F32, kind="ExternalInput")
    uk_in = nc.dram_tensor("uk", [H, 3 * H], F32, kind="ExternalInput")
    bkc_in = nc.dram_tensor("bkc", [BLK, 4], F32, kind="ExternalInput")
    w1_in = nc.dram_tensor("w1", [H, RU], F32, kind="ExternalInput")
    w2_in = nc.dram_tensor("w2", [RU, RU], F32, kind="ExternalInput")
    w3_in = nc.dram_tensor("w3", [RU, 1], F32, kind="ExternalInput")
    b1r_in = nc.dram_tensor("b1r", [BLK, 2], F32, kind="ExternalInput")
    b1e_in = nc.dram_tensor("b1e", [BLK, 2], F32, kind="ExternalInput")
    b2r_in = nc.dram_tensor("b2r", [BLK, 2], F32, kind="ExternalInput")
    b2e_in = nc.dram_tensor("b2e", [BLK, 2], F32, kind="ExternalInput")
    out_dram = nc.dram_tensor("out", [1, G], F32, kind="ExternalOutput")
    dbg_dram = (
        nc.dram_tensor("dbg_hT", [BLK, SHP], F32, kind="ExternalOutput")
        if DEBUG_HT
        else None
    )
    dbg_agg = (
        nc.dram_tensor("dbg_aggT", [BLK, SHP], F32, kind="ExternalOutput")
        if DEBUG_HT
        else None
    )

    with tile.TileContext(nc) as tc:
        with (
            tc.tile_pool(name="const", bufs=1) as cp,
            tc.tile_pool(name="work", bufs=3) as wp,
            tc.tile_pool(name="bigwork", bufs=2) as bwp,
            tc.tile_pool(name="final", bufs=2) as fp,
            tc.tile_pool(name="ps_small", bufs=4, space="PSUM") as psS,
            tc.tile_pool(name="ps_msg", bufs=2, space="PSUM") as psM,
            tc.tile_pool(name="ps_agg", bufs=2, space="PSUM") as psA,
            tc.tile_pool(name="dram", bufs=1, space="DRAM") as dp,
        ):
            # --- resident tiles ---
            ident = cp.tile([BLK, BLK], F32)
            make_identity(nc, ident[:])
            iota_i = cp.tile([BLK, BLK], I32)
            nc.gpsimd.iota(iota_i[:], pattern=[[1, BLK]], base=0, channel_multiplier=0)
            iota_f = cp.tile([BLK, BLK], F32)
            nc.vector.tensor_copy(iota_f[:], iota_i[:])
            iotc_i = cp.tile([BLK, 1], I32)
            nc.gpsimd.iota(iotc_i[:], pattern=[[1, 1]], base=0, channel_multiplier=1)
            iota_col = cp.tile([BLK, 1], F32)
            nc.vector.tensor_copy(iota_col[:], iotc_i[:])
            iog_i = cp.tile([BLK, G], I32)
            nc.gpsimd.iota(iog_i[:], pattern=[[1, G]], base=0, channel_multiplier=0)
            iota_g = cp.tile([BLK, G], F32)
            nc.vector.tensor_copy(iota_g[:], iog_i[:])
            ones1 = cp.tile([1, BLK], F32)
            nc.gpsimd.memset(ones1[:], 1.0)
            lnla_c = cp.tile([BLK, 1], F32)
            nc.gpsimd.memset(lnla_c[:], LNLA)

            hT = cp.tile([BLK, SHP], F32)
            nc.sync.dma_start(hT[:], h0T_in[:])
            gi_sb = cp.tile([BLK, NBLK * tpb], I32)
            nc.sync.dma_start(gi_sb[:], gi_in[:])
            segc_sb = cp.tile([BLK, NBLK * tpb], F32)
            nc.sync.dma_start(segc_sb[:], segc_in[:])
            gid_sb = cp.tile([BLK, NBLK], F32)
            nc.sync.dma_start(gid_sb[:], gid_in[:])

            wm1 = cp.tile([H, H], F32)
            nc.sync.dma_start(wm1[:], wm1_in[:])
            wm2 = cp.tile([H, H], F32)
            nc.sync.dma_start(wm2[:], wm2_in[:])
            bmr = cp.tile([1, H], F32)
            nc.sync.dma_start(bmr[:], bmr_in[:])
            wk = cp.tile([H, 3 * H], F32)
            nc.sync.dma_start(wk[:], wk_in[:])
            uk = cp.tile([H, 3 * H], F32)
            nc.sync.dma_start(uk[:], uk_in[:])
            bkc = cp.tile([BLK, 4], F32)
            nc.sync.dma_start(bkc[:], bkc_in[:])
            w1 = cp.tile([H, RU], F32)
            nc.sync.dma_start(w1[:], w1_in[:])
            w2aa = cp.tile([BLK, BLK], F32)
            nc.sync.dma_start(w2aa[:], w2_in[0:BLK, 0:BLK])
            w2ab = cp.tile([BLK, BLK], F32)
            nc.sync.dma_start(w2ab[:], w2_in[0:BLK, BLK:RU])
            w2ba = cp.tile([BLK, BLK], F32)
            nc.sync.dma_start(w2ba[:], w2_in[BLK:RU, 0:BLK])
            w2bb = cp.tile([BLK, BLK], F32)
            nc.sync.dma_start(w2bb[:], w2_in[BLK:RU, BLK:RU])
            w3a = cp.tile([BLK, 1], F32)
            nc.sync.dma_start(w3a[:], w3_in[0:BLK, :])
            w3b = cp.tile([BLK, 1], F32)
            nc.sync.dma_start(w3b[:], w3_in[BLK:RU, :])
            b1r = cp.tile([BLK, 2], F32)
            nc.sync.dma_start(b1r[:], b1r_in[:])
            b1e = cp.tile([BLK, 2], F32)
            nc.sync.dma_start(b1e[:], b1e_in[:])
            b2r = cp.tile([BLK, 2], F32)
            nc.sync.dma_start(b2r[:], b2r_in[:])
            b2e = cp.tile([BLK, 2], F32)
            nc.sync.dma_start(b2e[:], b2e_in[:])
            b3c = cp.tile([1, 1], F32)
            nc.gpsimd.memset(b3c[:], float(b3_val))

            # --- DRAM scratch ---
            repA = dp.tile([N_CORES * SHP, H], F32)
            repB = dp.tile([N_CORES * SHP, H], F32)
            shard_out = dp.tile([SHP, H], F32)
            pool_in = dp.tile([BLK, G], F32)
            pool_out = dp.tile([BLK, G], F32)

            def block_body(b, src_ap, t):
                # walrus requires a static (physical) offset AP for indirect
                # DMA: copy this block's indices to a fixed tile first.
                # HW consumes ONE offset per partition per call, so issue one
                # indirect gather per 128-message tile.
                idx_blk = wp.tile([BLK, tpb], I32, tag="idx_blk")
                nc.vector.tensor_copy(idx_blk[:], gi_sb[:, bass.ts(b, tpb)])
                gmain = bwp.tile([BLK, tpb * H], F32, tag="gmain")
                if "gather" not in ABLATE:
                    for u in range(1 if "gather1" in ABLATE else tpb):
                        nc.gpsimd.indirect_dma_start(
                            out=gmain[:, bass.ts(u, H)],
                            out_offset=None,
                            in_=src_ap,
                            in_offset=bass.IndirectOffsetOnAxis(
                                ap=idx_blk[:, u : u + 1], axis=0
                            ),
                        )
                # indT[s, m] = (seg[m] == s) for all tpb tiles of this block
                srow = bwp.tile([1, tpb * BLK], F32, tag="srow")
                nc.sync.dma_start(srow[:], segr_in[:1, bass.ts(b, tpb * BLK)])
                segb = bwp.tile([BLK, tpb * BLK], F32, tag="segb")
                nc.gpsimd.partition_broadcast(segb[:], srow[:1, :])
                indT = bwp.tile([BLK, tpb * BLK], F32, tag="indT")
                nc.vector.tensor_scalar(
                    out=indT[:], in0=segb[:], scalar1=iota_col[:, :1], scalar2=None,
                    op0=mybir.AluOpType.is_equal,
                )
                # static-offset copy of this block of hT (walrus: no register
                # offsets on ldweights operands)
                h_in = wp.tile([BLK, BLK], F32, tag="h_in")
                nc.vector.tensor_copy(h_in[:], hT[:, bass.ts(b, BLK)])
                # C = h_blk @ Wm2 + bm  (node-term of the message MLP)
                c_ps = psS.tile([BLK, H], F32, space="PSUM", tag="ps")
                nc.tensor.matmul(c_ps[:], lhsT=h_in[:], rhs=wm2[:],
                                 start=True, stop=False)
                nc.tensor.matmul(c_ps[:], lhsT=ones1[:], rhs=bmr[:],
                                 start=False, stop=True)
                c_sb = wp.tile([BLK, H], F32, tag="c_sb")
                nc.scalar.copy(c_sb[:], c_ps[:])

                aggT_ps = psA.tile([BLK, BLK], F32, space="PSUM", tag="aggT")
                for u in range(1) if "msg1" in ABLATE else ([] if "msg" in ABLATE else range(tpb)):
                    mt_ps = psS.tile([BLK, BLK], F32, space="PSUM", tag="ps")
                    nc.tensor.transpose(mt_ps[:], gmain[:, bass.ts(u, H)], ident[:])
                    mainT = wp.tile([BLK, BLK], F32, tag="mainT")
                    nc.vector.tensor_copy(mainT[:], mt_ps[:])
                    msg_ps = psM.tile([BLK, H], F32, space="PSUM", tag="msg")
                    nc.tensor.matmul(msg_ps[:], lhsT=mainT[:], rhs=wm1[:],
                                     start=True, stop=False)
                    nc.tensor.matmul(msg_ps[:], lhsT=indT[:, bass.ts(u, BLK)],
                                     rhs=c_sb[:], start=False, stop=True)
                    r_t = wp.tile([BLK, H], F32, tag="r_t")
                    nc.scalar.activation(
                        r_t[:], msg_ps[:], mybir.ActivationFunctionType.Relu,
                        scale=LAM,
                    )
                    e2_t = wp.tile([BLK, H], F32, tag="e2_t")
                    nc.scalar.activation(
                        e2_t[:], msg_ps[:], mybir.ActivationFunctionType.Exp,
                        bias=lnla_c[:, :1], scale=1.0,
                    )
                    b_t = wp.tile([BLK, H], F32, tag="b_t")
                    nc.vector.tensor_scalar(
                        out=b_t[:], in0=e2_t[:], scalar1=LA, scalar2=0.0,
                        op0=mybir.AluOpType.subtract, op1=mybir.AluOpType.min,
                    )
                    ind_ms = wp.tile([BLK, BLK], F32, tag="ind_ms")
                    nc.vector.tensor_scalar(
                        out=ind_ms[:], in0=iota_f[:],
                        scalar1=segc_sb[:, bass.ds(b * tpb + u, 1)], scalar2=None,
                        op0=mybir.AluOpType.is_equal,
                    )
                    # aggT[j, s] += r.T-free matmul: lhsT=r [m, j], rhs=ind [m, s]
                    nc.tensor.matmul(aggT_ps[:], lhsT=r_t[:], rhs=ind_ms[:],
                                     start=(u == 0), stop=False)
                    nc.tensor.matmul(aggT_ps[:], lhsT=b_t[:], rhs=ind_ms[:],
                                     start=False, stop=(u == tpb - 1))

                aggT = wp.tile([BLK, BLK], F32, tag="aggT_sb")
                nc.scalar.copy(aggT[:], aggT_ps[:])
                if dbg_agg is not None and t == 0:
                    nc.sync.dma_start(dbg_agg[:, bass.ts(b, BLK)], aggT[:])

                # --- GRU (feature-transposed layout [l, n]) ---
                h_blk = h_in[:]
                mz_ps = psS.tile([BLK, BLK], F32, space="PSUM", tag="ps")
                nc.tensor.matmul(mz_ps[:], lhsT=wk[:, 0:H], rhs=aggT[:],
                                 start=True, stop=False)
                nc.tensor.matmul(mz_ps[:], lhsT=uk[:, 0:H], rhs=h_blk,
                                 start=False, stop=True)
                zT = wp.tile([BLK, BLK], F32, tag="zT")
                nc.scalar.activation(zT[:], mz_ps[:],
                                     mybir.ActivationFunctionType.Sigmoid,
                                     bias=bkc[:, 0:1])
                mr_ps = psS.tile([BLK, BLK], F32, space="PSUM", tag="ps")
                nc.tensor.matmul(mr_ps[:], lhsT=wk[:, H : 2 * H], rhs=aggT[:],
                                 start=True, stop=False)
                nc.tensor.matmul(mr_ps[:], lhsT=uk[:, H : 2 * H], rhs=h_blk,
                                 start=False, stop=True)
                rT = wp.tile([BLK, BLK], F32, tag="rT")
                nc.scalar.activation(rT[:], mr_ps[:],
                                     mybir.ActivationFunctionType.Sigmoid,
                                     bias=bkc[:, 1:2])
                mhx_ps = psS.tile([BLK, BLK], F32, space="PSUM", tag="ps")
                nc.tensor.matmul(mhx_ps[:], lhsT=wk[:, 2 * H : 3 * H], rhs=aggT[:],
                                 start=True, stop=True)
                mhh_ps = psS.tile([BLK, BLK], F32, space="PSUM", tag="ps")
                nc.tensor.matmul(mhh_ps[:], lhsT=uk[:, 2 * H : 3 * H], rhs=h_blk,
                                 start=True, stop=True)
                t1 = wp.tile([BLK, BLK], F32, tag="t1")
                nc.vector.tensor_scalar(out=t1[:], in0=mhh_ps[:],
                                        scalar1=bkc[:, 3:4], scalar2=None,
                                        op0=mybir.AluOpType.add)
                t2 = wp.tile([BLK, BLK], F32, tag="t2")
                nc.vector.tensor_tensor(out=t2[:], in0=t1[:], in1=rT[:],
                                        op=mybir.AluOpType.mult)
                t3 = wp.tile([BLK, BLK], F32, tag="t3")
                nc.vector.tensor_tensor(out=t3[:], in0=t2[:], in1=mhx_ps[:],
                                        op=mybir.AluOpType.add)
                hhT = wp.tile([BLK, BLK], F32, tag="hhT")
                nc.scalar.activation(hhT[:], t3[:],
                                     mybir.ActivationFunctionType.Tanh,
                                     bias=bkc[:, 2:3])
                d_t = wp.tile([BLK, BLK], F32, tag="d_t")
                nc.vector.tensor_tensor(out=d_t[:], in0=h_blk, in1=hhT[:],
                                        op=mybir.AluOpType.subtract)
                e_t = wp.tile([BLK, BLK], F32, tag="e_t")
                nc.vector.tensor_tensor(out=e_t[:], in0=zT[:], in1=d_t[:],
                                        op=mybir.AluOpType.mult)
                hnT = wp.tile([BLK, BLK], F32, tag="hnT")
                nc.vector.tensor_tensor(out=hnT[:], in0=hhT[:], in1=e_t[:],
                                        op=mybir.AluOpType.add)
                nc.vector.tensor_copy(hT[:, bass.ts(b, BLK)], hnT[:])

                if t < T_ITERS - 1 and "stage" not in ABLATE:
                    hn_ps = psS.tile([BLK, BLK], F32, space="PSUM", tag="ps")
                    nc.tensor.transpose(hn_ps[:], hnT[:], ident[:])
                    hn_sb = wp.tile([BLK, BLK], F32, tag="hn_sb")
                    nc.scalar.copy(hn_sb[:], hn_ps[:])
                    nc.sync.dma_start(shard_out[bass.ts(b, BLK), :], hn_sb[:])

            # --- main iterations ---
            for t in range(T_ITERS):
                if t == 0:
                    src_ap = feat_sm[:]
                elif t % 2 == 1:
                    src_ap = repA[:]
                else:
                    src_ap = repB[:]
                with tc.For_i(
                    0, NBLK, UNROLL,
                    staggered_reset=STAGGERED,
                    hint_engines=HINTS,
                ) as b:
                    for db in range(UNROLL):
                        block_body(b + db if db else b, src_ap, t)
                if t < T_ITERS - 1 and "allgather" not in ABLATE:
                    dst = repA if t % 2 == 0 else repB
                    nc.gpsimd.collective_compute(
                        "AllGather",
                        mybir.AluOpType.bypass,
                        replica_groups=AG_GROUPS,
                        ins=[shard_out.opt()],
                        outs=[dst.opt()],
                    )

            if dbg_dram is not None:
                nc.sync.dma_start(dbg_dram[:], hT[:])

            # --- graph pooling: pooledT[j, g] = sum_s h[s, j] * (gid[s] == g) ---
            pool_ps = psM.tile([BLK, G], F32, space="PSUM", tag="msg")
            for b in range(NBLK):
                hb_ps = psS.tile([BLK, BLK], F32, space="PSUM", tag="ps")
                nc.tensor.transpose(hb_ps[:], hT[:, bass.ts(b, BLK)], ident[:])
                hb_sb = fp.tile([BLK, BLK], F32, tag="hb_sb")
                nc.scalar.copy(hb_sb[:], hb_ps[:])
                indg = fp.tile([BLK, G], F32, tag="indg")
                nc.vector.tensor_scalar(
                    out=indg[:], in0=iota_g[:], scalar1=gid_sb[:, b : b + 1],
                    scalar2=None, op0=mybir.AluOpType.is_equal,
                )
                nc.tensor.matmul(pool_ps[:], lhsT=hb_sb[:], rhs=indg[:],
                                 start=(b == 0), stop=(b == NBLK - 1))
            pooledT = fp.tile([BLK, G], F32, tag="pooledT")
            nc.vector.tensor_copy(pooledT[:], pool_ps[:])
            nc.sync.dma_start(pool_in[:], pooledT[:])
            nc.gpsimd.collective_compute(
                "AllReduce",
                mybir.AluOpType.add,
                replica_groups=AG_GROUPS,
                ins=[pool_in.opt()],
                outs=[pool_out.opt()],
            )
            pld = fp.tile([BLK, G], F32, tag="pld")
            nc.sync.dma_start(pld[:], pool_out[:])

            # --- MLP ---
            def selu_block(x_ps, brel_col, bexp_col, tagp):
                rr = fp.tile([BLK, G], F32, tag="f_r")
                nc.scalar.activation(rr[:], x_ps[:],
                                     mybir.ActivationFunctionType.Relu,
                                     bias=brel_col, scale=LAM)
                ee = fp.tile([BLK, G], F32, tag="f_e")
                nc.scalar.activation(ee[:], x_ps[:],
                                     mybir.ActivationFunctionType.Exp,
                                     bias=bexp_col, scale=1.0)
                bb = fp.tile([BLK, G], F32, tag="f_b")
                nc.vector.tensor_scalar(
                    out=bb[:], in0=ee[:], scalar1=LA, scalar2=0.0,
                    op0=mybir.AluOpType.subtract, op1=mybir.AluOpType.min,
                )
                oo = fp.tile([BLK, G], F32, tag=f"{tagp}_o")
                nc.vector.tensor_tensor(out=oo[:], in0=rr[:], in1=bb[:],
                                        op=mybir.AluOpType.add)
                return oo

            x1 = []
            for half in range(2):
                x_ps = psM.tile([BLK, G], F32, space="PSUM", tag="msg")
                nc.tensor.matmul(x_ps[:], lhsT=w1[:, bass.ts(half, BLK)], rhs=pld[:],
                                 start=True, stop=True)
                x1.append(selu_block(x_ps, b1r[:, half : half + 1],
                                     b1e[:, half : half + 1], f"x1{half}"))
            x2 = []
            w2t = [[w2aa, w2ab], [w2ba, w2bb]]
            for half in range(2):
                x_ps = psM.tile([BLK, G], F32, space="PSUM", tag="msg")
                nc.tensor.matmul(x_ps[:], lhsT=w2t[0][half][:], rhs=x1[0][:],
                                 start=True, stop=False)
                nc.tensor.matmul(x_ps[:], lhsT=w2t[1][half][:], rhs=x1[1][:],
                                 start=False, stop=True)
                x2.append(selu_block(x_ps, b2r[:, half : half + 1],
                                     b2e[:, half : half + 1], f"x2{half}"))
            x3_ps = psS.tile([1, G], F32, space="PSUM", tag="ps")
            nc.tensor.matmul(x3_ps[:], lhsT=w3a[:], rhs=x2[0][:],
                             start=True, stop=False)
            nc.tensor.matmul(x3_ps[:], lhsT=w3b[:], rhs=x2[1][:],
                             start=False, stop=True)
            out_sb = fp.tile([1, G], F32, tag="out_sb")
            nc.scalar.activation(out_sb[:], x3_ps[:],
                                 mybir.ActivationFunctionType.Identity,
                                 bias=b3c[:1, :1])
            nc.sync.dma_start(out_dram[:], out_sb[:])

    nc.compile()
    return nc


def kernel(features, edges_topology, graph_ids, Wm, bm, Wk, Uk, bk,
           W1, b1, W2, b2, W3, b3, _trace=False):
    features = np.asarray(features, np.float32)
    Wm = np.asarray(Wm, np.float32)
    bm = np.asarray(bm, np.float32)
    Wk = np.asarray(Wk, np.float32)
    Uk = np.asarray(Uk, np.float32)
    bk = np.asarray(bk, np.float32)
    W1 = np.asarray(W1, np.float32)
    b1 = np.asarray(b1, np.float32)
    W2 = np.asarray(W2, np.float32)
    b2 = np.asarray(b2, np.float32)
    W3 = np.asarray(W3, np.float32)
    b3 = np.asarray(b3, np.float32)
    et = np.asarray(edges_topology)

    pp = _preprocess(features, et[0], et[1], graph_ids)
    tpb = pp["tpb"]

    nc = _build_program(tpb, float(b3[0]))

    bkc = np.stack(
        [
            bk[0, 0:H] + bk[1, 0:H],
            bk[0, H : 2 * H] + bk[1, H : 2 * H],
            bk[0, 2 * H : 3 * H],
            bk[1, 2 * H : 3 * H],
        ],
        axis=1,
    ).astype(np.float32)  # [128, 4]: bz, br, bhx, bhh

    b1r = np.stack([LAM * b1[0:BLK], LAM * b1[BLK:RU]], axis=1).astype(np.float32)
    b1e = np.stack([b1[0:BLK] + LNLA, b1[BLK:RU] + LNLA], axis=1).astype(np.float32)
    b2r = np.stack([LAM * b2[0:BLK], LAM * b2[BLK:RU]], axis=1).astype(np.float32)
    b2e = np.stack([b2[0:BLK] + LNLA, b2[BLK:RU] + LNLA], axis=1).astype(np.float32)

    in_maps = []
    for c in range(N_CORES):
        in_maps.append(
            {
                "feat_sm": pp["feat_sm"],
                "h0T": pp["h0T"][c],
                "gi": pp["gi"][c],
                "segc": pp["segc"][c],
                "segr": pp["segr"][c],
                "gid": pp["gid"][c],
                "wm1": np.ascontiguousarray(Wm[0:H]),
                "wm2": np.ascontiguousarray(Wm[H : 2 * H]),
                "bmr": bm.reshape(1, H),
                "wk": Wk,
                "uk": Uk,
                "bkc": bkc,
                "w1": W1,
                "w2": W2,
                "w3": W3,
                "b1r": b1r,
                "b1e": b1e,
                "b2r": b2r,
                "b2e": b2e,
            }
        )

    res = bass_utils.run_bass_kernel_spmd(
        nc, in_maps, core_ids=list(range(N_CORES)), trace=_trace
    )
    out = res.results[0]["out"].reshape(G, 1).astype(np.float32)
    kernel.last_results = res
    return out
